# revision 13
# baseline (speedup 1.0000x reference)
"""GraphUNet Bass kernel for trn2 (8 NeuronCores, SPMD).

Strategy: row-shard A across 8 cores. Each core holds A[:, own] = (A[own, :]).T
(A symmetric) cast to bf16 resident in SBUF. The 4 graph-conv passes are
orientation-2 matmuls (stationary = activation tiles streamed from DRAM after
each AllGather, moving = A^T strips from SBUF), producing transposed outputs
that are PE-transposed back per 128-block.
Top-k pooling is done in full N-space with a mask (no gather):
  xu = m * relu((A@z + z) / (A@m + 1)),  z = (m*g*h0)@W1 + m*b1
Pass 1 runs fully in f32 (fused with the A load) so pooling scores match the
f32 reference; passes 2-4 use bf16 operands with f32 PSUM accumulation.
"""
import numpy as np
import concourse.bass as bass
import concourse.mybir as mybir
import concourse.tile as tile
from concourse import masks
from concourse.bass_utils import run_bass_kernel_spmd

NC = 8
N = 8192
R = N // NC          # 1024 rows per core
B = R // 128         # 8 own node blocks
KT = N // 128        # 64 global k-tiles
FIN = 256
H = 128
FOUT = 16
KSEL = 4096

F32 = mybir.dt.float32
BF16 = mybir.dt.bfloat16
AF = mybir.ActivationFunctionType
OP = mybir.AluOpType

RANK_CHUNK = 1024
NCHUNK = N // RANK_CHUNK


def _patch_tile_drain():
    """This container's walrus accepts at most one sync-wait per CTRL
    instruction; TileContext's tail drain collects one wait per
    outstanding DMA semaphore. Split the extras onto single-wait nops."""
    if getattr(tile.TileContext, "_drain_patched", False):
        return
    from concourse.tile import ScopedClock

    def _drain_and_barrier(self, tick_clock, wait_clock):
        nc = self.nc
        drain_inst = nc.sync.drain()
        wait_clock.add_sem_waits(
            drain_inst.ins, ScopedClock({None: tick_clock.global_clock})
        )
        si = drain_inst.ins.sync_info
        if si is not None and len(si.on_wait) > 1:
            waits = list(si.on_wait)
            drain_inst.ins.sync_info = mybir.SyncInfo(
                on_wait=[waits[0]], on_update=list(si.on_update)
            )
            for w in waits[1:]:
                nop = nc.sync.nop()
                nop.ins.sync_info = mybir.SyncInfo(on_wait=[w], on_update=[])
        nc.all_engine_barrier()
        popped = nc._tile_sem_poison_stack.pop()
        assert popped is self._sem_poison
        nc.clear_and_free_semaphores(list(self.sems.allocated().values()))
        nc.all_engine_barrier()

    tile.TileContext._drain_and_barrier = _drain_and_barrier
    tile.TileContext._drain_patched = True


def _split_multi_waits(nc):
    """Walrus in this container accepts one sync-wait per instruction.
    Move extra waits onto fresh single-wait NoOps inserted just before
    the owning instruction on the same engine (program-order equivalent)."""
    nsplit = 0
    for fn in nc.m.functions:
        for bb in fn.blocks:
            i = 0
            while i < len(bb.instructions):
                ins = bb.instructions[i]
                si = ins.sync_info
                if si is not None and len(si.on_wait) > 1:
                    waits = list(si.on_wait)
                    nops = []
                    for k, w in enumerate(waits[:-1]):
                        nop = mybir.InstNoOp(
                            name=f"{ins.name}-wsplit{k}", ins=[], outs=[]
                        )
                        nop.engine = ins.engine
                        nop.sync_info = mybir.SyncInfo(
                            on_wait=[w], on_update=[]
                        )
                        nops.append(nop)
                    ins.sync_info = mybir.SyncInfo(
                        on_wait=[waits[-1]], on_update=list(si.on_update)
                    )
                    bb.instructions[i:i] = nops
                    i += len(nops)
                    nsplit += 1
                i += 1
    return nsplit


def build_nc():
    _patch_tile_drain()
    nc = bass.Bass("TRN2", num_devices=NC)

    xc = nc.dram_tensor("xc", [R, FIN], F32, kind="ExternalInput")
    ac = nc.dram_tensor("ac", [N, R], BF16, kind="ExternalInput")
    W0 = nc.dram_tensor("W0", [FIN, H], F32, kind="ExternalInput")
    b0 = nc.dram_tensor("b0", [H], F32, kind="ExternalInput")
    W1 = nc.dram_tensor("W1", [H, H], F32, kind="ExternalInput")
    b1 = nc.dram_tensor("b1", [H], F32, kind="ExternalInput")
    pw = nc.dram_tensor("pw", [H, 1], F32, kind="ExternalInput")
    pb = nc.dram_tensor("pb", [1], F32, kind="ExternalInput")
    Wu = nc.dram_tensor("Wu", [H, H], F32, kind="ExternalInput")
    bu = nc.dram_tensor("bu", [H], F32, kind="ExternalInput")
    Wf = nc.dram_tensor("Wf", [H, FOUT], F32, kind="ExternalInput")
    bf = nc.dram_tensor("bf", [FOUT], F32, kind="ExternalInput")
    out = nc.dram_tensor("out", [R, FOUT], F32, kind="ExternalOutput")

    groups = [list(range(NC))]

    with tile.TileContext(nc) as tc:
        import contextlib

        ctx = contextlib.ExitStack()
        with ctx:
            const = ctx.enter_context(tc.tile_pool(name="const", bufs=1))
            big = ctx.enter_context(tc.tile_pool(name="big", bufs=1))
            work = ctx.enter_context(tc.tile_pool(name="work", bufs=1))
            small = ctx.enter_context(tc.tile_pool(name="small", bufs=2))
            psA = ctx.enter_context(tc.tile_pool(name="psA", bufs=2, space="PSUM"))
            psB = ctx.enter_context(tc.tile_pool(name="psB", bufs=2, space="PSUM"))
            psT = ctx.enter_context(tc.tile_pool(name="psT", bufs=2, space="PSUM"))
            dram = ctx.enter_context(tc.tile_pool(name="dram", bufs=1, space="DRAM"))

            # ---- constants ----
            ident_bf = const.tile([128, 128], BF16)
            masks.make_identity(nc, ident_bf[:])
            ident_f32 = const.tile([128, 128], F32)
            masks.make_identity(nc, ident_f32[:])
            ones_f32 = const.tile([128, 1], F32)
            nc.gpsimd.memset(ones_f32[:], 1.0)
            ones_bf = const.tile([128, 1], BF16)
            nc.gpsimd.memset(ones_bf[:], 1.0)
            onesrow = const.tile([1, 128], F32)
            nc.gpsimd.memset(onesrow[:], 1.0)

            W0_sb = const.tile([128, FIN], F32)  # 2 k-tiles [fin,128h]
            for t in range(2):
                nc.sync.dma_start(
                    W0_sb[:, t * 128 : (t + 1) * 128],
                    W0[t * 128 : (t + 1) * 128, :],
                )
            b0_sb = const.tile([128, 1], F32)
            nc.sync.dma_start(b0_sb[:], b0.rearrange("(p o) -> p o", o=1))
            bu_sb = const.tile([128, 1], F32)
            nc.sync.dma_start(bu_sb[:], bu.rearrange("(p o) -> p o", o=1))
            bf_sb = const.tile([16, 1], F32)
            nc.sync.dma_start(bf_sb[:], bf.rearrange("(p o) -> p o", o=1))

            wld = small.tile([128, H], F32, tag="wld")
            nc.sync.dma_start(wld[:], W1[:, :])
            W1_sb = const.tile([128, H], BF16)
            nc.vector.tensor_copy(W1_sb[:], wld[:])
            wld = small.tile([128, H], F32, tag="wld")
            nc.sync.dma_start(wld[:], Wu[:, :])
            Wu_sb = const.tile([128, H], BF16)
            nc.vector.tensor_copy(Wu_sb[:], wld[:])
            wld = small.tile([128, H], F32, tag="wld")
            nc.sync.dma_start(wld[:, 0:FOUT], Wf[:, :])
            Wf_sb = const.tile([128, FOUT], BF16)
            nc.vector.tensor_copy(Wf_sb[:], wld[:, 0:FOUT])

            b1row_f = small.tile([1, H], F32, tag="b1r")
            nc.sync.dma_start(b1row_f[:], b1.rearrange("(o h) -> o h", o=1))
            b1_row = const.tile([1, H], BF16)
            nc.vector.tensor_copy(b1_row[:], b1row_f[:])

            pwrow = small.tile([1, H], F32, tag="b1r")
            nc.sync.dma_start(pwrow[:], pw.rearrange("h o -> o h"))
            pw_bc = const.tile([128, H], F32)
            ptb0 = psT.tile([128, 128], F32, tag="tp")
            nc.tensor.matmul(ptb0[:], onesrow[:], pwrow[:], start=True, stop=True)
            nc.vector.tensor_copy(pw_bc[:], ptb0[:])
            pb_sb = small.tile([1, 1], F32, tag="pbs")
            nc.sync.dma_start(pb_sb[:], pb.rearrange("(o n) -> o n", o=1))
            pb_bc = const.tile([128, 1], F32)
            ptb1 = psT.tile([128, 128], F32, tag="tp")
            nc.tensor.matmul(ptb1[:, 0:1], onesrow[:], pb_sb[:], start=True, stop=True)
            nc.vector.tensor_copy(pb_bc[:], ptb1[:, 0:1])

            # ---- x transpose + P = x@W0 + b0 (all f32) ----
            xT = work.tile([128, 2 * R], F32, tag="t8k")  # [fin_t][128, R]
            for j in range(B):
                xst = small.tile([128, FIN], F32, tag="xst")
                nc.sync.dma_start(xst[:], xc[j * 128 : (j + 1) * 128, :])
                for t in range(2):
                    pt = psT.tile([128, 128], F32, tag="tp")
                    nc.tensor.matmul(
                        pt[:], xst[:, t * 128 : (t + 1) * 128], ident_f32[:],
                        start=True, stop=True,
                    )
                    nc.vector.tensor_copy(
                        xT[:, t * R + j * 128 : t * R + (j + 1) * 128], pt[:]
                    )
            psPT = psB.tile([128, R], F32, tag="pb")
            for t in range(2):
                for hh in range(2):
                    nc.tensor.matmul(
                        psPT[:, hh * 512 : (hh + 1) * 512],
                        W0_sb[:, t * 128 : (t + 1) * 128],
                        xT[:, t * R + hh * 512 : t * R + (hh + 1) * 512],
                        start=(t == 0), stop=(t == 1),
                    )
            PT_b = work.tile([128, R], F32, tag="ptb")
            nc.scalar.activation(PT_b[:], psPT[:], AF.Identity, bias=b0_sb[:, 0:1])
            P_own = work.tile([128, R], F32, tag="pown")
            for j in range(B):
                pt = psT.tile([128, 128], F32, tag="tp")
                nc.tensor.matmul(
                    pt[:], PT_b[:, j * 128 : (j + 1) * 128], ident_f32[:],
                    start=True, stop=True,
                )
                nc.vector.tensor_copy(P_own[:, j * 128 : (j + 1) * 128], pt[:])
            # split P into bf16 hi/lo (exact to ~2^-16) and AllGather the
            # packed pair; A is 0/1 so bf16 A-tiles are exact and pass 1
            # runs at bf16 rate with f32-grade accuracy.
            P_hi_own = work.tile([128, R], BF16, tag="phiown")
            nc.vector.tensor_copy(P_hi_own[:], P_own[:])
            hi_f = work.tile([128, R], F32, tag="t8k")
            nc.vector.tensor_copy(hi_f[:], P_hi_own[:])
            nc.vector.tensor_sub(hi_f[:], P_own[:], hi_f[:])
            P_lo_own = work.tile([128, R], BF16, tag="ploown")
            nc.vector.tensor_copy(P_lo_own[:], hi_f[:])
            agP_in = dram.tile([R, 2 * H], BF16)
            nc.sync.dma_start(
                agP_in.rearrange("(j p) f -> p j f", p=128)[:, :, 0:H],
                P_hi_own[:].rearrange("p (j f) -> p j f", f=H),
            )
            nc.sync.dma_start(
                agP_in.rearrange("(j p) f -> p j f", p=128)[:, :, H : 2 * H],
                P_lo_own[:].rearrange("p (j f) -> p j f", f=H),
            )
            agP_out = dram.tile([N, 2 * H], BF16, addr_space="Shared")
            nc.gpsimd.collective_compute(
                "AllGather", OP.bypass, replica_groups=groups,
                ins=[agP_in.opt()], outs=[agP_out.opt()],
            )

            # ---- A load (bf16, 2 queues, fully resident) + pass 1 + rowsum ----
            abuf = big.tile([128, KT * 1024], BF16)
            for k in range(KT):
                eng = nc.sync if k % 2 == 0 else nc.scalar
                eng.dma_start(
                    abuf[:, k * 1024 : (k + 1) * 1024],
                    ac[k * 128 : (k + 1) * 128, :],
                )
            racc = work.tile([128, 1024], BF16, tag="racc")
            L1 = psA.tile([128, 512], F32, tag="pa")
            R1 = psA.tile([128, 512], F32, tag="pa")
            for k in range(KT):
                at = abuf[:, k * 1024 : (k + 1) * 1024]
                pst = small.tile([128, 2 * H], BF16, tag="pstat", bufs=8)
                nc.sync.dma_start(pst[:], agP_out[k * 128 : (k + 1) * 128, :])
                if k == 0:
                    nc.vector.tensor_copy(racc[:], at)
                else:
                    nc.vector.tensor_add(racc[:], racc[:], at)
                nc.tensor.matmul(
                    L1[:], pst[:, 0:H], at[:, 0:512], start=(k == 0), stop=False,
                )
                nc.tensor.matmul(
                    R1[:], pst[:, 0:H], at[:, 512:1024], start=(k == 0), stop=False,
                )
                nc.tensor.matmul(
                    L1[:], pst[:, H : 2 * H], at[:, 0:512], start=False, stop=False,
                )
                nc.tensor.matmul(
                    R1[:], pst[:, H : 2 * H], at[:, 512:1024], start=False, stop=False,
                )
            for j in range(B):
                half, col = (L1, j * 128) if j < 4 else (R1, (j - 4) * 128)
                nc.tensor.matmul(
                    half[:, col : col + 128],
                    P_hi_own[:, j * 128 : (j + 1) * 128], ident_bf[:],
                    start=False, stop=False,
                )
                nc.tensor.matmul(
                    half[:, col : col + 128],
                    P_lo_own[:, j * 128 : (j + 1) * 128], ident_bf[:],
                    start=False, stop=(j == 3 or j == 7),
                )

            # rinv = 1/(rowsum+1) in [128, B] node layout
            psr = psB.tile([1, 1024], F32, tag="pb")
            for hh in range(2):
                nc.tensor.matmul(
                    psr[:, hh * 512 : (hh + 1) * 512],
                    ones_bf[:], racc[:, hh * 512 : (hh + 1) * 512],
                    start=True, stop=True,
                )
            rrow = small.tile([1, 1024], F32, tag="rrow", bufs=1)
            nc.scalar.copy(rrow[:], psr[:])
            rb = dram.tile([1024], F32)
            nc.sync.dma_start(rb.rearrange("(o n) -> o n", o=1), rrow[:])
            rinv_sb = const.tile([128, B], F32)
            nc.sync.dma_start(rinv_sb[:], rb.rearrange("(j p) -> p j", p=128))
            nc.vector.tensor_scalar_add(rinv_sb[:], rinv_sb[:], 1.0)
            nc.vector.reciprocal(rinv_sb[:], rinv_sb[:])

            # ---- pass 1 epilogue (f32): transpose + relu-scale; score ----
            uT1 = work.tile([128, 1024], F32, tag="t8k")
            nc.vector.tensor_copy(uT1[:, 0:512], L1[:])
            nc.scalar.copy(uT1[:, 512:1024], R1[:])
            h0bf = work.tile([128, R], BF16, tag="h0bf")
            s_sb = const.tile([128, B], F32)
            for j in range(B):
                pt = psT.tile([128, 128], F32, tag="tp")
                nc.tensor.matmul(
                    pt[:], uT1[:, j * 128 : (j + 1) * 128], ident_f32[:],
                    start=True, stop=True,
                )
                h0f = small.tile([128, 128], F32, tag="h0f")
                nc.scalar.activation(
                    h0f[:], pt[:], AF.Relu, scale=rinv_sb[:, j : j + 1]
                )
                nc.vector.tensor_copy(h0bf[:, j * 128 : (j + 1) * 128], h0f[:])
                scr = small.tile([128, 128], F32, tag="scr")
                nc.vector.tensor_mul(scr[:], h0f[:], pw_bc[:])
                nc.vector.tensor_reduce(
                    s_sb[:, j : j + 1], scr[:],
                    axis=mybir.AxisListType.X, op=OP.add,
                )
            nc.vector.tensor_scalar_add(s_sb[:], s_sb[:], pb_bc[:, 0:1])

            # ---- score allgather + ranking ----
            sc_d = dram.tile([1024], F32)
            nc.sync.dma_start(sc_d.rearrange("(j p) -> p j", p=128), s_sb[:])
            sg_d = dram.tile([N], F32, addr_space="Shared")
            nc.gpsimd.collective_compute(
                "AllGather", OP.bypass, replica_groups=groups,
                ins=[sc_d.opt()], outs=[sg_d.opt()],
            )

            # sum of squares -> 1/||s||  (in [128, KT] node layout)
            sgt = small.tile([128, KT], F32, tag="sgt")
            nc.sync.dma_start(sgt[:], sg_d.rearrange("(k p) -> p k", p=128))
            sqs = small.tile([128, KT], F32, tag="sqs")
            sqcol = small.tile([128, 1], F32, tag="sqc")
            nc.scalar.activation(sqs[:], sgt[:], AF.Square, accum_out=sqcol[:])
            ps_s = psT.tile([128, 128], F32, tag="tp")
            nc.tensor.matmul(
                ps_s[0:1, 0:1], ones_f32[:], sqcol[:], start=True, stop=True
            )
            ssq = small.tile([1, 1], F32, tag="ssq")
            nc.scalar.copy(ssq[:], ps_s[0:1, 0:1])
            rssq = small.tile([1, 1], F32, tag="rsq")
            nc.vector.reciprocal(rssq[:], ssq[:])
            invn = small.tile([1, 1], F32, tag="inv")
            nc.scalar.activation(invn[:], rssq[:], AF.Sqrt)
            invn_bc = const.tile([128, 1], F32)
            ptb2 = psT.tile([128, 128], F32, tag="tp")
            nc.tensor.matmul(ptb2[:, 0:1], onesrow[:], invn[:], start=True, stop=True)
            nc.vector.tensor_copy(invn_bc[:], ptb2[:, 0:1])

            # rank counting: cnt[i] = #{j : s_j > s_i}, split across the
            # vector (is_gt) and scalar (Sign) engines. Scalar chunks give
            # T = sum sign(s_j - s_i); #gt = (T + n)/2 up to a +0.5 bias
            # from the self-tie, which cannot flip the integer-threshold
            # comparison cnt < KSEL.
            cnt = const.tile([128, B], F32)
            Tacc = const.tile([128, B], F32)
            nso = small.tile([128, B], F32, tag="nso", bufs=1)
            nc.vector.tensor_scalar_mul(nso[:], s_sb[:], -1.0)
            for c in range(NCHUNK):
                sgr = small.tile([1, RANK_CHUNK], F32, tag="sgr", bufs=2)
                nc.sync.dma_start(
                    sgr[:],
                    sg_d[c * RANK_CHUNK : (c + 1) * RANK_CHUNK].rearrange(
                        "(o n) -> o n", o=1
                    ),
                )
                sgb = psB.tile([128, RANK_CHUNK], F32, tag="pb")
                for hh in range(RANK_CHUNK // 512):
                    nc.tensor.matmul(
                        sgb[:, hh * 512 : (hh + 1) * 512], onesrow[:],
                        sgr[:, hh * 512 : (hh + 1) * 512],
                        start=True, stop=True,
                    )
                for j in range(B):
                    if c % 2 == 0:
                        cmp = small.tile(
                            [128, RANK_CHUNK], BF16, tag="cmp", bufs=2
                        )
                        if c == 0:
                            nc.vector.tensor_scalar(
                                cmp[:], sgb[:], s_sb[:, j : j + 1], None,
                                OP.is_gt, op1=OP.add,
                                accum_out=cnt[:, j : j + 1],
                            )
                        else:
                            ctmp = small.tile([128, 1], F32, tag="ctmp", bufs=3)
                            nc.vector.tensor_scalar(
                                cmp[:], sgb[:], s_sb[:, j : j + 1], None,
                                OP.is_gt, op1=OP.add, accum_out=ctmp[:],
                            )
                            nc.vector.tensor_add(
                                cnt[:, j : j + 1], cnt[:, j : j + 1], ctmp[:]
                            )
                    else:
                        sgn = small.tile(
                            [128, RANK_CHUNK], BF16, tag="sgn", bufs=2
                        )
                        if c == 1:
                            nc.scalar.activation(
                                sgn[:], sgb[:], AF.Sign,
                                bias=nso[:, j : j + 1],
                                accum_out=Tacc[:, j : j + 1],
                            )
                        else:
                            ttmp = small.tile([128, 1], F32, tag="ttmp", bufs=3)
                            nc.scalar.activation(
                                sgn[:], sgb[:], AF.Sign,
                                bias=nso[:, j : j + 1], accum_out=ttmp[:],
                            )
                            nc.vector.tensor_add(
                                Tacc[:, j : j + 1], Tacc[:, j : j + 1], ttmp[:]
                            )
            # cnt += 0.5*T + (n_scalar_chunks * RANK_CHUNK)/2
            nc.vector.tensor_scalar(
                Tacc[:], Tacc[:], 0.5, float(NCHUNK // 2 * RANK_CHUNK // 2),
                OP.mult, op1=OP.add,
            )
            nc.vector.tensor_add(cnt[:], cnt[:], Tacc[:])

            m_f32 = const.tile([128, B], F32)
            nc.vector.tensor_scalar(
                m_f32[:], cnt[:], float(KSEL), None, OP.is_lt
            )
            g_sb = small.tile([128, B], F32, tag="gsb")
            nc.scalar.activation(
                g_sb[:], s_sb[:], AF.Sigmoid, scale=invn_bc[:, 0:1]
            )
            gm = const.tile([128, B], F32)
            nc.vector.tensor_mul(gm[:], m_f32[:], g_sb[:])
            m_bf = small.tile([128, B], BF16, tag="mbf")
            nc.vector.tensor_copy(m_bf[:], m_f32[:])
            mc_d = dram.tile([1024], BF16)
            nc.sync.dma_start(mc_d.rearrange("(j p) -> p j", p=128), m_bf[:])
            mg_d = dram.tile([N], BF16, addr_space="Shared")
            nc.gpsimd.collective_compute(
                "AllGather", OP.bypass, replica_groups=groups,
                ins=[mc_d.opt()], outs=[mg_d.opt()],
            )
            mg_sb = work.tile([128, KT], BF16, tag="mg")
            nc.sync.dma_start(mg_sb[:], mg_d.rearrange("(k p) -> p k", p=128))
            m_row = work.tile([1, 1024], BF16, tag="mrow")
            nc.sync.dma_start(m_row[:], mc_d.rearrange("(o n) -> o n", o=1))

            # ---- z = (gm*h0)@W1 + m*b1 ----
            yT = work.tile([128, R], BF16, tag="yT")
            for j in range(B):
                dg = small.tile([128, 128], BF16, tag="diag")
                nc.vector.tensor_scalar_mul(dg[:], ident_bf[:], gm[:, j : j + 1])
                pt = psT.tile([128, 128], F32, tag="tp")
                nc.tensor.matmul(
                    pt[:], h0bf[:, j * 128 : (j + 1) * 128], dg[:],
                    start=True, stop=True,
                )
                nc.vector.tensor_copy(yT[:, j * 128 : (j + 1) * 128], pt[:])
            psZ = psB.tile([128, R], F32, tag="pb")
            for hh in range(2):
                nc.tensor.matmul(
                    psZ[:, hh * 512 : (hh + 1) * 512], W1_sb[:],
                    yT[:, hh * 512 : (hh + 1) * 512], start=True, stop=False,
                )
                nc.tensor.matmul(
                    psZ[:, hh * 512 : (hh + 1) * 512], b1_row[:],
                    m_row[:, hh * 512 : (hh + 1) * 512], start=False, stop=True,
                )
            zT = work.tile([128, R], BF16, tag="zT")
            nc.vector.tensor_copy(zT[:], psZ[:])
            z_own = work.tile([128, R], BF16, tag="zown")
            for j in range(B):
                pt = psT.tile([128, 128], F32, tag="tp")
                nc.tensor.matmul(
                    pt[:], zT[:, j * 128 : (j + 1) * 128], ident_bf[:],
                    start=True, stop=True,
                )
                nc.vector.tensor_copy(z_own[:, j * 128 : (j + 1) * 128], pt[:])
            agZ_in = dram.tile([R, H], BF16)
            nc.sync.dma_start(
                agZ_in.rearrange("(j p) f -> p j f", p=128),
                z_own[:].rearrange("p (j f) -> p j f", f=H),
            )
            agZ_out = dram.tile([N, H], BF16, addr_space="Shared")
            nc.gpsimd.collective_compute(
                "AllGather", OP.bypass, replica_groups=groups,
                ins=[agZ_in.opt()], outs=[agZ_out.opt()],
            )

            # ---- pass 2: u2 = A@z + z ; rp = A@m + 1 ----
            L2 = psA.tile([128, 512], F32, tag="pa")
            R2 = psA.tile([128, 512], F32, tag="pa")
            psrp = psB.tile([1, 1024], F32, tag="pb")
            for k in range(KT):
                at = abuf[:, k * 1024 : (k + 1) * 1024]
                zst = small.tile([128, H], BF16, tag="zstat", bufs=8)
                nc.sync.dma_start(zst[:], agZ_out[k * 128 : (k + 1) * 128, :])
                nc.tensor.matmul(
                    L2[:], zst[:], at[:, 0:512], start=(k == 0), stop=False,
                )
                nc.tensor.matmul(
                    R2[:], zst[:], at[:, 512:1024], start=(k == 0), stop=False,
                )
                nc.tensor.matmul(
                    psrp[:, 0:512], mg_sb[:, k : k + 1], at[:, 0:512],
                    start=(k == 0), stop=(k == KT - 1),
                )
                nc.tensor.matmul(
                    psrp[:, 512:1024], mg_sb[:, k : k + 1], at[:, 512:1024],
                    start=(k == 0), stop=(k == KT - 1),
                )
            for j in range(B):
                half, col = (L2, j * 128) if j < 4 else (R2, (j - 4) * 128)
                nc.tensor.matmul(
                    half[:, col : col + 128],
                    z_own[:, j * 128 : (j + 1) * 128], ident_bf[:],
                    start=False, stop=(j == 3 or j == 7),
                )
            rprow = small.tile([1, 1024], F32, tag="rrow", bufs=1)
            nc.scalar.copy(rprow[:], psrp[:])
            rpb = dram.tile([1024], F32)
            nc.sync.dma_start(rpb.rearrange("(o n) -> o n", o=1), rprow[:])
            mrpinv = const.tile([128, B], F32)
            nc.sync.dma_start(mrpinv[:], rpb.rearrange("(j p) -> p j", p=128))
            nc.vector.tensor_scalar_add(mrpinv[:], mrpinv[:], 1.0)
            nc.vector.reciprocal(mrpinv[:], mrpinv[:])
            nc.vector.tensor_mul(mrpinv[:], m_f32[:], mrpinv[:])

            uT2 = work.tile([128, 1024], BF16, tag="t8k")
            nc.vector.tensor_copy(uT2[:, 0:512], L2[:])
            nc.scalar.copy(uT2[:, 512:1024], R2[:])
            xu = work.tile([128, R], BF16, tag="h0bf")
            for j in range(B):
                pt = psT.tile([128, 128], F32, tag="tp")
                nc.tensor.matmul(
                    pt[:], uT2[:, j * 128 : (j + 1) * 128], ident_bf[:],
                    start=True, stop=True,
                )
                nc.scalar.activation(
                    xu[:, j * 128 : (j + 1) * 128], pt[:], AF.Relu,
                    scale=mrpinv[:, j : j + 1],
                )

            # ---- P3 = xu@Wu + bu ----
            xuT = work.tile([128, R], BF16, tag="yT")
            for j in range(B):
                pt = psT.tile([128, 128], F32, tag="tp")
                nc.tensor.matmul(
                    pt[:], xu[:, j * 128 : (j + 1) * 128], ident_bf[:],
                    start=True, stop=True,
                )
                nc.vector.tensor_copy(xuT[:, j * 128 : (j + 1) * 128], pt[:])
            psP3 = psB.tile([128, R], F32, tag="pb")
            for hh in range(2):
                nc.tensor.matmul(
                    psP3[:, hh * 512 : (hh + 1) * 512], Wu_sb[:],
                    xuT[:, hh * 512 : (hh + 1) * 512], start=True, stop=True,
                )
            p3T = work.tile([128, R], BF16, tag="zT")
            nc.scalar.activation(p3T[:], psP3[:], AF.Identity, bias=bu_sb[:, 0:1])
            p3_own = work.tile([128, R], BF16, tag="zown")
            for j in range(B):
                pt = psT.tile([128, 128], F32, tag="tp")
                nc.tensor.matmul(
                    pt[:], p3T[:, j * 128 : (j + 1) * 128], ident_bf[:],
                    start=True, stop=True,
                )
                nc.vector.tensor_copy(p3_own[:, j * 128 : (j + 1) * 128], pt[:])
            ag3_in = dram.tile([R, H], BF16)
            nc.sync.dma_start(
                ag3_in.rearrange("(j p) f -> p j f", p=128),
                p3_own[:].rearrange("p (j f) -> p j f", f=H),
            )
            ag3_out = dram.tile([N, H], BF16, addr_space="Shared")
            nc.gpsimd.collective_compute(
                "AllGather", OP.bypass, replica_groups=groups,
                ins=[ag3_in.opt()], outs=[ag3_out.opt()],
            )

            # ---- pass 3 ----
            L3 = psA.tile([128, 512], F32, tag="pa")
            R3 = psA.tile([128, 512], F32, tag="pa")
            for k in range(KT):
                at = abuf[:, k * 1024 : (k + 1) * 1024]
                zst = small.tile([128, H], BF16, tag="zstat", bufs=8)
                nc.sync.dma_start(zst[:], ag3_out[k * 128 : (k + 1) * 128, :])
                nc.tensor.matmul(
                    L3[:], zst[:], at[:, 0:512], start=(k == 0), stop=False,
                )
                nc.tensor.matmul(
                    R3[:], zst[:], at[:, 512:1024], start=(k == 0), stop=False,
                )
            for j in range(B):
                half, col = (L3, j * 128) if j < 4 else (R3, (j - 4) * 128)
                nc.tensor.matmul(
                    half[:, col : col + 128],
                    p3_own[:, j * 128 : (j + 1) * 128], ident_bf[:],
                    start=False, stop=(j == 3 or j == 7),
                )
            uT3 = work.tile([128, 1024], BF16, tag="t8k")
            nc.vector.tensor_copy(uT3[:, 0:512], L3[:])
            nc.scalar.copy(uT3[:, 512:1024], R3[:])
            hu = work.tile([128, R], BF16, tag="h0bf")
            for j in range(B):
                pt = psT.tile([128, 128], F32, tag="tp")
                nc.tensor.matmul(
                    pt[:], uT3[:, j * 128 : (j + 1) * 128], ident_bf[:],
                    start=True, stop=True,
                )
                nc.scalar.activation(
                    hu[:, j * 128 : (j + 1) * 128], pt[:], AF.Relu,
                    scale=rinv_sb[:, j : j + 1],
                )

            # ---- P4 = hu@Wf + bf ----
            huT = work.tile([128, R], BF16, tag="yT")
            for j in range(B):
                pt = psT.tile([128, 128], F32, tag="tp")
                nc.tensor.matmul(
                    pt[:], hu[:, j * 128 : (j + 1) * 128], ident_bf[:],
                    start=True, stop=True,
                )
                nc.vector.tensor_copy(huT[:, j * 128 : (j + 1) * 128], pt[:])
            psP4 = psB.tile([16, R], F32, tag="pb")
            for hh in range(2):
                nc.tensor.matmul(
                    psP4[:, hh * 512 : (hh + 1) * 512], Wf_sb[:],
                    huT[:, hh * 512 : (hh + 1) * 512], start=True, stop=True,
                )
            p4T = work.tile([16, R], BF16, tag="p4T")
            nc.scalar.activation(p4T[:], psP4[:], AF.Identity, bias=bf_sb[:, 0:1])
            p4_own = work.tile([128, B * FOUT], BF16, tag="p4own")
            for j in range(B):
                pt = psT.tile([128, 128], F32, tag="tp")
                nc.tensor.matmul(
                    pt[:, 0:FOUT], p4T[:, j * 128 : (j + 1) * 128],
                    ident_bf[:16, :16], start=True, stop=True,
                )
                nc.vector.tensor_copy(
                    p4_own[:, j * FOUT : (j + 1) * FOUT], pt[:, 0:FOUT]
                )
            ag4_in = dram.tile([R, FOUT], BF16)
            nc.sync.dma_start(
                ag4_in.rearrange("(j p) f -> p j f", p=128),
                p4_own[:].rearrange("p (j f) -> p j f", f=FOUT),
            )
            ag4_out = dram.tile([N, FOUT], BF16, addr_space="Shared")
            nc.gpsimd.collective_compute(
                "AllGather", OP.bypass, replica_groups=groups,
                ins=[ag4_in.opt()], outs=[ag4_out.opt()],
            )

            # ---- pass 4 + log_softmax ----
            L4 = psA.tile([16, 512], F32, tag="pa")
            R4 = psA.tile([16, 512], F32, tag="pa")
            for k in range(KT):
                at = abuf[:, k * 1024 : (k + 1) * 1024]
                p4st = small.tile([128, FOUT], BF16, tag="p4stat", bufs=8)
                nc.sync.dma_start(p4st[:], ag4_out[k * 128 : (k + 1) * 128, :])
                nc.tensor.matmul(
                    L4[:], p4st[:], at[:, 0:512], start=(k == 0), stop=False,
                )
                nc.tensor.matmul(
                    R4[:], p4st[:], at[:, 512:1024], start=(k == 0), stop=False,
                )
            for j in range(B):
                half, col = (L4, j * 128) if j < 4 else (R4, (j - 4) * 128)
                nc.tensor.matmul(
                    half[:, col : col + 128],
                    p4_own[:, j * FOUT : (j + 1) * FOUT], ident_bf[:],
                    start=False, stop=(j == 3 or j == 7),
                )
            u4T = work.tile([16, 1024], F32, tag="t8k")
            nc.vector.tensor_copy(u4T[:, 0:512], L4[:])
            nc.scalar.copy(u4T[:, 512:1024], R4[:])
            out_sb = work.tile([128, B * FOUT], F32, tag="osb")
            for j in range(B):
                pt = psT.tile([128, 128], F32, tag="tp")
                nc.tensor.matmul(
                    pt[:, 0:FOUT], u4T[:, j * 128 : (j + 1) * 128],
                    ident_f32[:16, :16], start=True, stop=True,
                )
                fsb = small.tile([128, FOUT], F32, tag="fsb")
                nc.scalar.activation(
                    fsb[:], pt[:, 0:FOUT], AF.Identity,
                    scale=rinv_sb[:, j : j + 1],
                )
                mx = small.tile([128, 1], F32, tag="mx")
                nc.vector.tensor_reduce(
                    mx[:], fsb[:], axis=mybir.AxisListType.X, op=OP.max
                )
                nmx = small.tile([128, 1], F32, tag="nmx")
                nc.vector.tensor_scalar_mul(nmx[:], mx[:], -1.0)
                esc = small.tile([128, FOUT], F32, tag="esc")
                se = small.tile([128, 1], F32, tag="se")
                nc.scalar.activation(
                    esc[:], fsb[:], AF.Exp, bias=nmx[:, 0:1], accum_out=se[:]
                )
                lse = small.tile([128, 1], F32, tag="lse")
                nc.scalar.activation(lse[:], se[:], AF.Ln)
                nc.vector.tensor_scalar(
                    out_sb[:, j * FOUT : (j + 1) * FOUT], fsb[:],
                    nmx[:, 0:1], lse[:, 0:1], OP.add, OP.subtract,
                )
            nc.sync.dma_start(
                out.rearrange("(j p) f -> p j f", p=128),
                out_sb[:].rearrange("p (j f) -> p j f", f=FOUT),
            )
    _split_multi_waits(nc)
    return nc


_NC_CACHE = None


def kernel(**inputs):
    global _NC_CACHE
    if _NC_CACHE is None:
        _NC_CACHE = build_nc()
    nc = _NC_CACHE
    import ml_dtypes

    x = np.asarray(inputs["x"], np.float32)
    A = np.asarray(inputs["A"], np.float32)
    shared = {
        k: np.asarray(inputs[k], np.float32)
        for k in ["W0", "b0", "W1", "b1", "pw", "pb", "Wu", "bu", "Wf", "bf"]
    }
    in_maps = []
    for c in range(NC):
        m = dict(shared)
        m["xc"] = np.ascontiguousarray(x[c * R : (c + 1) * R, :])
        # A is a 0/1 adjacency: bf16 is exact, halves the HBM read.
        m["ac"] = np.ascontiguousarray(
            A[:, c * R : (c + 1) * R]
        ).astype(ml_dtypes.bfloat16)
        in_maps.append(m)
    res = run_bass_kernel_spmd(nc, in_maps, list(range(NC)))
    return np.concatenate([res.results[c]["out"] for c in range(NC)], axis=0)

